# revision 6
# baseline (speedup 1.0000x reference)
"""Gumbel top-k (sequential masking) Trainium2 kernel, v5.

B=64 rows, N=16384, K=16 sequential top-1+mask steps; outputs st
(one-hot) and softs, each [K, B, N] f32 (softs emitted bf16, st u8).
Data-parallel: 8 rows/core x 8 cores; row = 16 partitions x 1024.
DRAM outputs partition-major; host transposes back.

v5 redesign vs v4:
  - selection (max/find) runs on z directly (argmax invariant under
    exp), overlapping the exp computation on ACT.
  - all 16 1/S_j scales from ONE tensor_tensor_scan + reciprocal
    (replaces two 3-step log-prefix chains on gpsimd).
  - mr tree shortened to e4 -> e8 -> e12; group-3 planes approximate
    like the others, fixed by DRAM scatter items packed into the pad
    slots of scatter B (no extra indirect op).
  - per-plane output DMAs; each group's exact plane (j%4==0) is
    produced LAST on its engine so fix-up scatters only wait on the
    fixup planes' DMAs and overlap the final exact-plane DMA.
  - gathers + offset machinery issued early on gpsimd; item columns
    are plain rank-pointer copies, +16 premult adjustment applied to
    the readback values instead of 11 column ops.
  - st zero source memset moved to the (otherwise idle) ACT engine;
    dummy activation up front prefetches the ACT table during the
    input DMA.
  - planes: DVE 8 (tensor_scalar mult, ~0.65us), ACT 6 (act copy,
    ~1.24us), GPS 2.
"""

import numpy as np
from contextlib import ExitStack

import concourse.bacc as bacc
import concourse.bass as bass
import concourse.mybir as mybir
import concourse.tile as tile
from concourse.bass import AP
from concourse.bass_utils import run_bass_kernel_spmd

F32 = mybir.dt.float32
BF16 = mybir.dt.bfloat16
U8 = mybir.dt.uint8
U32 = mybir.dt.uint32
AF = mybir.ActivationFunctionType
OP = mybir.AluOpType

B, N, NCORES = 64, 16384, 8
R = B // NCORES
QP = 16
FREE = N // QP           # 1024
P = 128
H = FREE // 2
INV_TAU = 1.5
K16 = 16
KF = K16 * FREE
SW = 80                  # staging row width (u32 elements per partition)
NEG = -1.0e30

# q-packed scatter items: (plane, rank) per q slot.  Pads duplicate the
# last real item (writing 0 twice is idempotent).
ITEMS_A = [(1, 0), (2, 0), (2, 1), (3, 0), (3, 1), (3, 2),
           (5, 4), (6, 4), (6, 5), (7, 4), (7, 5), (7, 6),
           (7, 6), (7, 6), (7, 6), (7, 6)]
ITEMS_B = [(9, 8), (10, 8), (10, 9), (11, 8), (11, 9), (11, 10),
           (13, 12), (14, 12), (14, 13), (15, 12), (15, 13), (15, 14),
           (15, 14), (15, 14), (15, 14), (15, 14)]

# engine per plane: 'act' | 'dve' | 'gps'
PLANE_ENG = {0: 'act', 1: 'act', 2: 'act', 3: 'act',
             4: 'dve', 5: 'dve', 6: 'gps', 7: 'gps',
             8: 'act', 9: 'act', 10: 'dve', 11: 'dve',
             12: 'dve', 13: 'dve', 14: 'dve', 15: 'dve'}

_module_cache = {}


def _host_consts():
    p = np.arange(P)
    q = p % 16
    cc = np.zeros((P, 8), np.uint32)
    cc[:, 0] = p * 16384                 # premult16384 half0 base
    cc[:, 1] = p * 16384 + 512           # premult16384 half1 base
    cc[:, 2] = (p // 16) * 1280          # slotflat row base (SW=80)
    cc[:, 3] = q * 1024                  # st plane offset (plane = q)
    ja = np.array([j for j, _ in ITEMS_A], np.uint32)        # slo-local
    jb = np.array([j - 8 for j, _ in ITEMS_B], np.uint32)    # shi-local
    cc[:, 4] = ja[q] * 1024
    cc[:, 5] = jb[q] * 1024
    cc[:, 6] = p * 8192                  # premult8192 half0 base
    cc[:, 7] = p * 8192 + 512            # premult8192 half1 base
    return cc


def _item_col_copies(items):
    """Compress the q -> rank mapping into copies.  Returns tuples
    (dst_start, src_col, width, is_broadcast); rank r lives at staged
    col 32+r.  Increasing-by-1 runs become strided copies, constant
    runs (the pads) become broadcast copies."""
    out = []
    qs = 0
    while qs < 16:
        r0 = items[qs][1]
        wi = 1
        while qs + wi < 16 and items[qs + wi][1] == r0 + wi:
            wi += 1
        wc = 1
        while qs + wc < 16 and items[qs + wc][1] == r0:
            wc += 1
        if wc > wi:
            out.append((qs, 32 + r0, wc, True))
            qs += wc
        else:
            out.append((qs, 32 + r0, wi, False))
            qs += wi
    return out


def _build16():
    nc = bacc.Bacc("TRN2", target_bir_lowering=False, debug=False,
                   num_devices=NCORES)
    zc_d = nc.dram_tensor("zc", [P, FREE + 8], F32, kind="ExternalInput")
    slo_d = nc.dram_tensor("slo", [P * KF // 2, 1], BF16,
                           kind="ExternalOutput")
    shi_d = nc.dram_tensor("shi", [P * KF // 2, 1], BF16,
                           kind="ExternalOutput")
    st_d = nc.dram_tensor("st", [P * KF, 1], U8, kind="ExternalOutput")
    stg_d = nc.dram_tensor("stg", [P * SW, 1], U32, kind="Internal")

    slo_2d = slo_d.ap().rearrange("(p f) o -> p (f o)", p=P)
    shi_2d = shi_d.ap().rearrange("(p f) o -> p (f o)", p=P)
    st_2d = st_d.ap().rearrange("(p f) o -> p (f o)", p=P)
    stg_2d = stg_d.ap().rearrange("(p c) o -> p (c o)", p=P)

    # strided diagonal readbacks: partition p=(16r+q) reads element
    # base + 1280r + 81q  (= p*80 + base + q)
    def diag_ap(base):
        return AP(stg_d.ap().tensor, base, [[1280, 8], [81, 16]])

    with tile.TileContext(nc) as tc, ExitStack() as ctx:
        sp = ctx.enter_context(tc.tile_pool(name="sp", bufs=1))

        zext = sp.tile([P, FREE + 8], F32, tag="zext")
        z = zext[:, 0:FREE]
        cc = zext[:, FREE:FREE + 8].bitcast(U32)
        e0 = sp.tile([P, FREE], F32, tag="e0")
        etiles = {0: e0}
        for t in (4, 8, 12):
            etiles[t] = sp.tile([P, FREE], F32, tag=f"e{t}", name=f"e{t}")
        softs_sb = sp.tile([P, KF], BF16, tag="softs_sb")
        stz = sp.tile([P, KF // 4], F32, tag="stz")
        selz = sp.tile([P, 18], F32, tag="selz")
        miu = sp.tile([P, 16], U32, tag="miu")
        scomb = sp.tile([P, SW], U32, tag="scomb")
        cand = sp.tile([P, 16 * 18], F32, tag="cand")
        vbr = sp.tile([P, 34], F32, tag="vbr")
        ec = sp.tile([P, 256], F32, tag="ec")
        c2 = sp.tile([P, 256], F32, tag="c2")
        ekeys = sp.tile([P, 16], F32, tag="ekeys")
        padk = sp.tile([P, 24], F32, tag="padk")
        negt = sp.tile([P, 16], F32, tag="negt")
        SSp = sp.tile([P, 16], F32, tag="SSp")
        slots = sp.tile([P, 16], U32, tag="slots")
        sadj = sp.tile([P, 64], U32, tag="sadj")
        rb = sp.tile([P, 3], U32, tag="rb")       # readback diag results
        rba = sp.tile([P, 2], U32, tag="rba")     # +16 adjusted A/B ptrs
        ob = sp.tile([P, 3], U32, tag="ob")       # gathered premult values
        otmp = sp.tile([P, 3], U32, tag="otmp")
        offd = sp.tile([P, 3], U32, tag="offd")
        ones = sp.tile([P, 1], U8, tag="ones")
        zbf = sp.tile([P, 1], BF16, tag="zbf")
        dum = sp.tile([P, 1], F32, tag="dum")

        # ---- phase 0: pre-input work -------------------------------
        # ACT: zero the st source block, then a dummy Exp to pull the
        # activation table in during the input DMA.
        nc.scalar.memzero(stz[:])
        nc.scalar.activation(dum[:], stz[:, 0:1], AF.Exp, scale=1.0)
        nc.vector.memset(padk[:], -1.0)
        nc.gpsimd.memset(ones[:], 1)
        nc.gpsimd.memset(zbf[:], 0.0)

        # input halves; consts ride with half B
        nc.sync.dma_start(out=zext[:, 0:H], in_=zc_d.ap()[:, 0:H])
        nc.sync.dma_start(out=zext[:, H:FREE + 8],
                          in_=zc_d.ap()[:, H:FREE + 8])
        # st zero-fill (reads stz after ACT memzero)
        nc.sync.dma_start(out=st_2d, in_=stz[:].bitcast(U8))

        # ---- phase 1: per-partition selection on z + exp -----------
        nc.vector.max(selz[:, 0:8], z[:, 0:H])
        nc.vector.max_index(miu[:, 0:8], selz[:, 0:8], z[:, 0:H])
        nc.vector.max(selz[:, 8:16], z[:, H:FREE])
        nc.vector.max_index(miu[:, 8:16], selz[:, 8:16], z[:, H:FREE])
        nc.scalar.activation(e0[:, 0:H], z[:, 0:H], AF.Exp, scale=INV_TAU,
                             accum_out=selz[:, 16:17])
        nc.scalar.activation(e0[:, H:FREE], z[:, H:FREE], AF.Exp,
                             scale=INV_TAU, accum_out=selz[:, 17:18])
        # gps: premult candidate offsets
        nc.gpsimd.tensor_tensor(scomb[:, 0:8], miu[:, 0:8],
                                cc[:, 0:1].to_broadcast([P, 8]), OP.add)
        nc.gpsimd.tensor_tensor(scomb[:, 8:16], miu[:, 8:16],
                                cc[:, 1:2].to_broadcast([P, 8]), OP.add)
        nc.gpsimd.tensor_tensor(scomb[:, 16:24], miu[:, 0:8],
                                cc[:, 6:7].to_broadcast([P, 8]), OP.add)
        nc.gpsimd.tensor_tensor(scomb[:, 24:32], miu[:, 8:16],
                                cc[:, 7:8].to_broadcast([P, 8]), OP.add)

        # ---- phase 2: row-level selection (DVE) --------------------
        for q in range(QP):
            nc.vector.stream_shuffle(cand[:, 18 * q:18 * q + 18], selz[:],
                                     [q] * 16 + [16 + q] * 16)
        gv = cand[:].rearrange("p (q c) -> p q c", c=18)
        nc.vector.tensor_reduce(vbr[:, 32:33], gv[:, :, 16:18],
                                axis=mybir.AxisListType.XY, op=OP.add)
        nc.vector.reciprocal(vbr[:, 16:17], vbr[:, 32:33])   # 1/S0
        nc.vector.max(vbr[:, 0:8], gv[:, :, 0:16])           # z-top 0-7
        nc.vector.tensor_copy(ec[:].rearrange("p (q j) -> p q j", j=16),
                              gv[:, :, 0:16])
        nc.vector.max_index(slots[:, 0:8], vbr[:, 0:8], ec[:])
        # ACT: e-space keys for ranks 0-7 (bit-identical to e0's values)
        nc.scalar.activation(ekeys[:, 0:8], vbr[:, 0:8], AF.Exp,
                             scale=INV_TAU)
        nc.vector.match_replace(c2[:], vbr[:, 0:8], ec[:], NEG)
        nc.vector.max(vbr[:, 8:16], c2[:])                   # z-top 8-15
        nc.vector.max_index(slots[:, 8:16], vbr[:, 8:16], c2[:])
        nc.scalar.activation(ekeys[:, 8:16], vbr[:, 8:16], AF.Exp,
                             scale=INV_TAU)

        # scales: SSp[:, j] = S_{j+1} = S0 - sum_{r<=j} etop_r
        nc.vector.tensor_scalar(negt[:], ekeys[:], -1.0, None, OP.mult)
        nc.vector.tensor_tensor_scan(SSp[:], negt[:], negt[:],
                                     vbr[:, 32:33], OP.add, OP.bypass)
        nc.vector.reciprocal(vbr[:, 17:32], SSp[:, 0:15])    # 1/S_1..15

        # mr tree keys (padk prefilled -1)
        nc.vector.tensor_copy(padk[:, 0:4], ekeys[:, 0:4])
        nc.vector.tensor_copy(padk[:, 8:12], ekeys[:, 4:8])
        nc.vector.tensor_copy(padk[:, 16:20], ekeys[:, 8:12])

        # slot -> staged flat pointer math (DVE, fast small ops)
        nc.vector.tensor_scalar(sadj[:, 0:16], slots[:], 5, None, OP.mult)
        nc.vector.tensor_scalar(sadj[:, 16:32], slots[:], 15, None,
                                OP.bitwise_and)
        nc.vector.tensor_scalar(sadj[:, 32:48], sadj[:, 16:32], 2, None,
                                OP.logical_shift_left)
        nc.vector.tensor_tensor(sadj[:, 48:64], sadj[:, 0:16],
                                sadj[:, 32:48], OP.subtract)
        nc.vector.tensor_tensor(scomb[:, 32:48], sadj[:, 48:64],
                                cc[:, 2:3].to_broadcast([P, 16]), OP.add)

        # item columns: plain copies of rank pointers (gps)
        for base, items in ((48, ITEMS_A), (64, ITEMS_B)):
            for dst, src, w, bc in _item_col_copies(items):
                src_ap = (scomb[:, src:src + 1].to_broadcast([P, w]) if bc
                          else scomb[:, src:src + w])
                nc.gpsimd.tensor_copy(scomb[:, base + dst:base + dst + w],
                                      src_ap)

        # ---- staging + ordered readbacks ---------------------------
        nc.sync.dma_start(out=stg_2d, in_=scomb[:])
        nc.gpsimd.memset(scomb[:, 79:80], 0)      # waits staging read
        for x in range(3):
            nc.gpsimd.tensor_copy(rb[:, x:x + 1], scomb[:, 79:80])
        nc.sync.dma_start(out=rb[:, 0:1], in_=diag_ap(32))   # st
        nc.sync.dma_start(out=rb[:, 1:2], in_=diag_ap(48))   # A
        nc.sync.dma_start(out=rb[:, 2:3], in_=diag_ap(64))   # B
        # +16: redirect A/B pointers to the premult8192 copies
        nc.gpsimd.tensor_scalar(rba[:], rb[:, 1:3], 16, None, OP.add)
        nc.gpsimd.indirect_dma_start(
            out=ob[:, 0:1], out_offset=None, in_=stg_d.ap(),
            in_offset=bass.IndirectOffsetOnAxis(ap=rb[:, 0:1], axis=0))
        nc.gpsimd.indirect_dma_start(
            out=ob[:, 1:2], out_offset=None, in_=stg_d.ap(),
            in_offset=bass.IndirectOffsetOnAxis(ap=rba[:, 0:1], axis=0))
        nc.gpsimd.indirect_dma_start(
            out=ob[:, 2:3], out_offset=None, in_=stg_d.ap(),
            in_offset=bass.IndirectOffsetOnAxis(ap=rba[:, 1:2], axis=0))
        nc.gpsimd.tensor_tensor(otmp[:, 0:1], ob[:, 0:1], cc[:, 3:4],
                                OP.add)
        nc.gpsimd.tensor_tensor(otmp[:, 1:2], ob[:, 1:2], cc[:, 4:5],
                                OP.add)
        nc.gpsimd.tensor_tensor(otmp[:, 2:3], ob[:, 2:3], cc[:, 5:6],
                                OP.add)

        # ---- mr tree (DVE) + planes + per-plane DMAs ---------------
        nc.vector.match_replace(etiles[4][:], padk[:, 0:8], e0[:], 0.0)
        nc.vector.match_replace(etiles[8][:], padk[:, 8:16], etiles[4][:],
                                0.0)
        nc.vector.match_replace(etiles[12][:], padk[:, 16:24],
                                etiles[8][:], 0.0)

        def plane(j):
            src = etiles[4 * (j // 4)]
            dst = softs_sb[:, j * FREE:(j + 1) * FREE]
            scl = vbr[:, 16 + j:17 + j]
            eng = PLANE_ENG[j]
            if eng == 'act':
                nc.scalar.activation(dst, src[:], AF.Copy, scale=scl)
            elif eng == 'dve':
                nc.vector.tensor_scalar(dst, src[:], scl, None, OP.mult)
            else:
                nc.gpsimd.tensor_scalar(dst, src[:], scl, None, OP.mult)

        def plane_dma(j):
            tgt = slo_2d if j < 8 else shi_2d
            toff = (j % 8) * FREE
            nc.sync.dma_start(out=tgt[:, toff:toff + FREE],
                              in_=softs_sb[:, j * FREE:(j + 1) * FREE])

        # fixup planes first per engine; exact plane (j%4==0) last.
        for j in (1, 2, 3, 0,            # ACT g0
                  5, 4,                  # DVE g1 part
                  6, 7,                  # GPS g1 part
                  9, 8,                  # ACT g2 part
                  10, 11,                # DVE g2 part
                  13, 14, 15, 12):       # DVE g3 (12 exact, last)
            plane(j)
            plane_dma(j)

        # ---- indirect scatters (gps), in token-readiness order -----
        # st: waits zero-fill DMA
        nc.gpsimd.memset(stz[:, 0:1], 0.0)
        nc.gpsimd.tensor_tensor(offd[:, 0:1], otmp[:, 0:1],
                                stz[:, 0:1].bitcast(U32), OP.add)
        nc.gpsimd.indirect_dma_start(
            out=st_d.ap(),
            out_offset=bass.IndirectOffsetOnAxis(ap=offd[:, 0:1], axis=0),
            in_=ones[:], in_offset=None)
        # A: waits fixup-plane DMAs 1,2,3,5,6,7
        for j in (1, 2, 3, 5, 6, 7):
            nc.gpsimd.memset(softs_sb[:, j * FREE:j * FREE + 2], 0.0)
        tokA = [softs_sb[:, j * FREE:j * FREE + 2].bitcast(U32)
                for j in (1, 2, 3, 5, 6, 7)]
        nc.gpsimd.tensor_tensor(offd[:, 1:2], otmp[:, 1:2], tokA[0],
                                OP.add)
        for t in tokA[1:]:
            nc.gpsimd.tensor_tensor(offd[:, 1:2], offd[:, 1:2], t, OP.add)
        nc.gpsimd.indirect_dma_start(
            out=slo_d.ap(),
            out_offset=bass.IndirectOffsetOnAxis(ap=offd[:, 1:2], axis=0),
            in_=zbf[:], in_offset=None)
        # B: waits fixup-plane DMAs 9,10,11,13,14,15 (NOT 12)
        for j in (9, 10, 11, 13, 14, 15):
            nc.gpsimd.memset(softs_sb[:, j * FREE:j * FREE + 2], 0.0)
        tokB = [softs_sb[:, j * FREE:j * FREE + 2].bitcast(U32)
                for j in (9, 10, 11, 13, 14, 15)]
        nc.gpsimd.tensor_tensor(offd[:, 2:3], otmp[:, 2:3], tokB[0],
                                OP.add)
        for t in tokB[1:]:
            nc.gpsimd.tensor_tensor(offd[:, 2:3], offd[:, 2:3], t, OP.add)
        nc.gpsimd.indirect_dma_start(
            out=shi_d.ap(),
            out_offset=bass.IndirectOffsetOnAxis(ap=offd[:, 2:3], axis=0),
            in_=zbf[:], in_offset=None)
    nc.compile()
    return nc


def kernel(logits, gumbel, k, trace=False):
    K = int(k)
    logits = np.ascontiguousarray(logits, dtype=np.float32)
    gumbel = np.ascontiguousarray(gumbel, dtype=np.float32)
    if K == 0:
        empty = np.zeros((0, B, N), dtype=np.float32)
        return empty, empty.copy()
    assert K == 16, f"kernel supports k=16 only, got {K}"
    assert logits.shape == (B, N) and gumbel.shape == (B, N)

    if K not in _module_cache:
        _module_cache[K] = _build16()
    nc = _module_cache[K]

    cc = _host_consts().view(np.float32)
    z_full = logits + gumbel
    in_maps = []
    for c in range(NCORES):
        sl = slice(c * R, (c + 1) * R)
        zc = np.concatenate([z_full[sl].reshape(P, FREE), cc], axis=1)
        in_maps.append({"zc": np.ascontiguousarray(zc)})

    res = run_bass_kernel_spmd(nc, in_maps, core_ids=list(range(NCORES)),
                               trace=trace)

    st = np.empty((K, B, N), dtype=np.float32)
    softs = np.empty((K, B, N), dtype=np.float32)
    for c in range(NCORES):
        sl = slice(c * R, (c + 1) * R)
        lo = res.results[c]["slo"].reshape(R, QP, 8, FREE)
        hi = res.results[c]["shi"].reshape(R, QP, 8, FREE)
        s = np.concatenate([lo, hi], axis=2)
        softs[:, sl, :] = np.transpose(s.astype(np.float32), (2, 0, 1, 3)) \
            .reshape(K, R, N)
        t = res.results[c]["st"].reshape(R, QP, K16, FREE)
        st[:, sl, :] = np.transpose(t, (2, 0, 1, 3)).reshape(K16, R, N) \
            .astype(np.float32)

    if trace:
        kernel.last_exec_time_ns = res.exec_time_ns
        kernel.last_results = res
    return st, softs


# revision 11
# speedup vs baseline: 1.5470x; 1.5470x over previous
"""Gumbel top-k (sequential masking) Trainium2 kernel, v5.

B=64 rows, N=16384, K=16 sequential top-1+mask steps; outputs st
(one-hot) and softs, each [K, B, N] f32 (softs emitted bf16, st u8).
Data-parallel: 8 rows/core x 8 cores; row = 16 partitions x 1024.
DRAM outputs partition-major; host transposes back.

v5 redesign vs v4:
  - selection (max/find) runs on z directly (argmax invariant under
    exp), overlapping the exp computation on ACT.
  - all 16 1/S_j scales from ONE tensor_tensor_scan + reciprocal
    (replaces two 3-step log-prefix chains on gpsimd).
  - mr tree shortened to e4 -> e8 -> e12; group-3 planes approximate
    like the others, fixed by DRAM scatter items packed into the pad
    slots of scatter B (no extra indirect op).
  - per-plane output DMAs; each group's exact plane (j%4==0) is
    produced LAST on its engine so fix-up scatters only wait on the
    fixup planes' DMAs and overlap the final exact-plane DMA.
  - gathers + offset machinery issued early on gpsimd; item columns
    are plain rank-pointer copies, +16 premult adjustment applied to
    the readback values instead of 11 column ops.
  - st zero source memset moved to the (otherwise idle) ACT engine;
    dummy activation up front prefetches the ACT table during the
    input DMA.
  - planes: DVE 8 (tensor_scalar mult, ~0.65us), ACT 6 (act copy,
    ~1.24us), GPS 2.
"""

import numpy as np
from contextlib import ExitStack

import concourse.bacc as bacc
import concourse.bass as bass
import concourse.mybir as mybir
import concourse.tile as tile
from concourse.bass import AP
from concourse.bass_utils import run_bass_kernel_spmd

F32 = mybir.dt.float32
BF16 = mybir.dt.bfloat16
U8 = mybir.dt.uint8
U32 = mybir.dt.uint32
AF = mybir.ActivationFunctionType
OP = mybir.AluOpType

B, N, NCORES = 64, 16384, 8
R = B // NCORES
QP = 16
FREE = N // QP           # 1024
P = 128
H = FREE // 2
INV_TAU = 1.5
K16 = 16
KF = K16 * FREE
SW = 80                  # staging row width (u32 elements per partition)
NEG = -1.0e30

# q-packed scatter items: (plane, rank) per q slot.  Pads duplicate the
# last real item (writing 0 twice is idempotent).
ITEMS_A = [(1, 0), (2, 0), (2, 1), (3, 0), (3, 1), (3, 2),
           (5, 4), (6, 4), (6, 5), (7, 4), (7, 5), (7, 6),
           (7, 6), (7, 6), (7, 6), (7, 6)]
ITEMS_B = [(9, 8), (10, 8), (10, 9), (11, 8), (11, 9), (11, 10),
           (13, 12), (14, 12), (14, 13), (15, 12), (15, 13), (15, 14),
           (15, 14), (15, 14), (15, 14), (15, 14)]

# engine per plane: 'act' | 'dve'  (gpsimd is ~10x too slow for wide
# tensor ops and starves DVE's SBUF ports while streaming)
PLANE_ENG = {0: 'act', 1: 'act', 2: 'act', 3: 'act',
             4: 'dve', 5: 'dve', 6: 'dve', 7: 'dve',
             8: 'act', 9: 'act', 10: 'dve', 11: 'dve',
             12: 'dve', 13: 'dve', 14: 'dve', 15: 'dve'}

_module_cache = {}


def _host_consts():
    p = np.arange(P)
    q = p % 16
    cc = np.zeros((P, 8), np.uint32)
    cc[:, 0] = p * 16384                 # premult16384 half0 base
    cc[:, 1] = p * 16384 + 512           # premult16384 half1 base
    cc[:, 2] = (p // 16) * 1280          # slotflat row base (SW=80)
    cc[:, 3] = q * 1024                  # st plane offset (plane = q)
    ja = np.array([j for j, _ in ITEMS_A], np.uint32)        # slo-local
    jb = np.array([j - 8 for j, _ in ITEMS_B], np.uint32)    # shi-local
    cc[:, 4] = ja[q] * 1024
    cc[:, 5] = jb[q] * 1024
    cc[:, 6] = p * 8192                  # premult8192 half0 base
    cc[:, 7] = p * 8192 + 512            # premult8192 half1 base
    return cc


def _item_col_copies(items):
    """Compress the q -> rank mapping into copies.  Returns tuples
    (dst_start, src_col, width, is_broadcast); rank r lives at staged
    col 32+r.  Increasing-by-1 runs become strided copies, constant
    runs (the pads) become broadcast copies."""
    out = []
    qs = 0
    while qs < 16:
        r0 = items[qs][1]
        wi = 1
        while qs + wi < 16 and items[qs + wi][1] == r0 + wi:
            wi += 1
        wc = 1
        while qs + wc < 16 and items[qs + wc][1] == r0:
            wc += 1
        if wc > wi:
            out.append((qs, 32 + r0, wc, True))
            qs += wc
        else:
            out.append((qs, 32 + r0, wi, False))
            qs += wi
    return out


def _build16():
    nc = bacc.Bacc("TRN2", target_bir_lowering=False, debug=False,
                   num_devices=NCORES)
    zc_d = nc.dram_tensor("zc", [P, FREE + 8], F32, kind="ExternalInput")
    slo_d = nc.dram_tensor("slo", [P * KF // 2, 1], BF16,
                           kind="ExternalOutput")
    shi_d = nc.dram_tensor("shi", [P * KF // 2, 1], BF16,
                           kind="ExternalOutput")
    st_d = nc.dram_tensor("st", [P * KF, 1], U8, kind="ExternalOutput")
    stg_d = nc.dram_tensor("stg", [P * SW, 1], U32, kind="Internal")

    slo_2d = slo_d.ap().rearrange("(p f) o -> p (f o)", p=P)
    shi_2d = shi_d.ap().rearrange("(p f) o -> p (f o)", p=P)
    st_2d = st_d.ap().rearrange("(p f) o -> p (f o)", p=P)
    stg_2d = stg_d.ap().rearrange("(p c) o -> p (c o)", p=P)

    # strided diagonal readbacks: partition p=(16r+q) reads element
    # base + 1280r + 81q  (= p*80 + base + q)
    def diag_ap(base):
        return AP(stg_d.ap().tensor, base, [[1280, 8], [81, 16]])

    with tile.TileContext(nc) as tc, ExitStack() as ctx:
        sp = ctx.enter_context(tc.tile_pool(name="sp", bufs=1))

        zext = sp.tile([P, FREE + 8], F32, tag="zext")
        z = zext[:, 0:FREE]
        cc = zext[:, FREE:FREE + 8].bitcast(U32)
        e0 = sp.tile([P, FREE], F32, tag="e0")
        etiles = {0: e0}
        for t in (4, 8, 12):
            etiles[t] = sp.tile([P, FREE], F32, tag=f"e{t}", name=f"e{t}")
        softs_sb = sp.tile([P, KF], BF16, tag="softs_sb")
        stz = sp.tile([P, KF // 4], F32, tag="stz")
        selz = sp.tile([P, 18], F32, tag="selz")
        miu = sp.tile([P, 16], U32, tag="miu")
        scomb = sp.tile([P, SW], U32, tag="scomb")
        cand = sp.tile([P, 16 * 18], F32, tag="cand")
        vbr = sp.tile([P, 34], F32, tag="vbr")
        ec = sp.tile([P, 256], F32, tag="ec")
        c2 = sp.tile([P, 256], F32, tag="c2")
        ekeys = sp.tile([P, 16], F32, tag="ekeys")
        padk = sp.tile([P, 24], F32, tag="padk")
        negt = sp.tile([P, 16], F32, tag="negt")
        SSp = sp.tile([P, 16], F32, tag="SSp")
        slots = sp.tile([P, 16], U32, tag="slots")
        sadj = sp.tile([P, 64], U32, tag="sadj")
        rb = sp.tile([P, 3], U32, tag="rb")       # readback diag results
        rba = sp.tile([P, 2], U32, tag="rba")     # +16 adjusted A/B ptrs
        ob = sp.tile([P, 3], U32, tag="ob")       # gathered premult values
        otmp = sp.tile([P, 3], U32, tag="otmp")
        offd = sp.tile([P, 3], U32, tag="offd")
        ones = sp.tile([P, 1], U8, tag="ones")
        zbf = sp.tile([P, 1], BF16, tag="zbf")

        # ---- phase 0: pre-input work -------------------------------
        # GPS zeroes the st source block (it has slack until the
        # staging phase).  The ACT table load fires at engine start on
        # its own, no dummy needed.
        nc.gpsimd.memset(stz[:], 0.0)
        nc.vector.memset(padk[:], -1.0)
        nc.gpsimd.memset(ones[:], 1)
        nc.gpsimd.memset(zbf[:], 0.0)

        # input halves; consts ride with half B
        nc.sync.dma_start(out=zext[:, 0:H], in_=zc_d.ap()[:, 0:H])
        nc.sync.dma_start(out=zext[:, H:FREE + 8],
                          in_=zc_d.ap()[:, H:FREE + 8])
        # st zero-fill (reads stz after ACT memzero)
        nc.sync.dma_start(out=st_2d, in_=stz[:].bitcast(U8))

        # ---- phase 1: per-partition selection on z + exp -----------
        nc.vector.max(selz[:, 0:8], z[:, 0:H])
        nc.vector.max_index(miu[:, 0:8], selz[:, 0:8], z[:, 0:H])
        nc.vector.max(selz[:, 8:16], z[:, H:FREE])
        nc.vector.max_index(miu[:, 8:16], selz[:, 8:16], z[:, H:FREE])
        nc.scalar.activation(e0[:, 0:H], z[:, 0:H], AF.Exp, scale=INV_TAU,
                             accum_out=selz[:, 16:17])
        nc.scalar.activation(e0[:, H:FREE], z[:, H:FREE], AF.Exp,
                             scale=INV_TAU, accum_out=selz[:, 17:18])
        # gps: premult candidate offsets
        nc.gpsimd.tensor_tensor(scomb[:, 0:8], miu[:, 0:8],
                                cc[:, 0:1].to_broadcast([P, 8]), OP.add)
        nc.gpsimd.tensor_tensor(scomb[:, 8:16], miu[:, 8:16],
                                cc[:, 1:2].to_broadcast([P, 8]), OP.add)
        nc.gpsimd.tensor_tensor(scomb[:, 16:24], miu[:, 0:8],
                                cc[:, 6:7].to_broadcast([P, 8]), OP.add)
        nc.gpsimd.tensor_tensor(scomb[:, 24:32], miu[:, 8:16],
                                cc[:, 7:8].to_broadcast([P, 8]), OP.add)

        # ---- phase 2: row-level selection (DVE) --------------------
        for q in range(QP):
            nc.vector.stream_shuffle(cand[:, 18 * q:18 * q + 18], selz[:],
                                     [q] * 16 + [16 + q] * 16)
        gv = cand[:].rearrange("p (q c) -> p q c", c=18)
        nc.vector.tensor_reduce(vbr[:, 32:33], gv[:, :, 16:18],
                                axis=mybir.AxisListType.XY, op=OP.add)
        nc.vector.reciprocal(vbr[:, 16:17], vbr[:, 32:33])   # 1/S0
        nc.vector.max(vbr[:, 0:8], gv[:, :, 0:16])           # z-top 0-7
        nc.vector.tensor_copy(ec[:].rearrange("p (q j) -> p q j", j=16),
                              gv[:, :, 0:16])
        nc.vector.max_index(slots[:, 0:8], vbr[:, 0:8], ec[:])
        # ACT: e-space keys for ranks 0-7 (bit-identical to e0's values)
        nc.scalar.activation(ekeys[:, 0:8], vbr[:, 0:8], AF.Exp,
                             scale=INV_TAU)
        nc.vector.match_replace(c2[:], vbr[:, 0:8], ec[:], NEG)
        nc.vector.max(vbr[:, 8:16], c2[:])                   # z-top 8-15
        nc.vector.max_index(slots[:, 8:16], vbr[:, 8:16], c2[:])
        nc.scalar.activation(ekeys[:, 8:16], vbr[:, 8:16], AF.Exp,
                             scale=INV_TAU)

        # scales: SSp[:, j] = S_{j+1} = S0 - sum_{r<=j} etop_r
        nc.vector.tensor_scalar(negt[:], ekeys[:], -1.0, None, OP.mult)
        nc.vector.tensor_tensor_scan(SSp[:], negt[:], negt[:],
                                     vbr[:, 32:33], OP.add, OP.bypass)
        nc.vector.reciprocal(vbr[:, 17:32], SSp[:, 0:15])    # 1/S_1..15

        # mr tree keys (padk prefilled -1)
        nc.vector.tensor_copy(padk[:, 0:4], ekeys[:, 0:4])
        nc.vector.tensor_copy(padk[:, 8:12], ekeys[:, 4:8])
        nc.vector.tensor_copy(padk[:, 16:20], ekeys[:, 8:12])

        # slot -> staged flat pointer math (DVE, fast small ops)
        nc.vector.tensor_scalar(sadj[:, 0:16], slots[:], 5, None, OP.mult)
        nc.vector.tensor_scalar(sadj[:, 16:32], slots[:], 15, None,
                                OP.bitwise_and)
        nc.vector.tensor_scalar(sadj[:, 32:48], sadj[:, 16:32], 2, None,
                                OP.logical_shift_left)
        nc.vector.tensor_tensor(sadj[:, 48:64], sadj[:, 0:16],
                                sadj[:, 32:48], OP.subtract)
        nc.vector.tensor_tensor(scomb[:, 32:48], sadj[:, 48:64],
                                cc[:, 2:3].to_broadcast([P, 16]), OP.add)

        # item columns: plain copies of rank pointers (gps)
        for base, items in ((48, ITEMS_A), (64, ITEMS_B)):
            for dst, src, w, bc in _item_col_copies(items):
                src_ap = (scomb[:, src:src + 1].to_broadcast([P, w]) if bc
                          else scomb[:, src:src + w])
                nc.gpsimd.tensor_copy(scomb[:, base + dst:base + dst + w],
                                      src_ap)

        # ---- staging + ordered readbacks ---------------------------
        nc.sync.dma_start(out=stg_2d, in_=scomb[:])
        nc.gpsimd.memset(scomb[:, 79:80], 0)      # waits staging read
        for x in range(3):
            nc.gpsimd.tensor_copy(rb[:, x:x + 1], scomb[:, 79:80])
        nc.sync.dma_start(out=rb[:, 0:1], in_=diag_ap(32))   # st
        nc.sync.dma_start(out=rb[:, 1:2], in_=diag_ap(48))   # A
        nc.sync.dma_start(out=rb[:, 2:3], in_=diag_ap(64))   # B
        # +16: redirect A/B pointers to the premult8192 copies
        nc.gpsimd.tensor_scalar(rba[:], rb[:, 1:3], 16, None, OP.add)
        nc.gpsimd.indirect_dma_start(
            out=ob[:, 0:1], out_offset=None, in_=stg_d.ap(),
            in_offset=bass.IndirectOffsetOnAxis(ap=rb[:, 0:1], axis=0))
        nc.gpsimd.indirect_dma_start(
            out=ob[:, 1:2], out_offset=None, in_=stg_d.ap(),
            in_offset=bass.IndirectOffsetOnAxis(ap=rba[:, 0:1], axis=0))
        nc.gpsimd.indirect_dma_start(
            out=ob[:, 2:3], out_offset=None, in_=stg_d.ap(),
            in_offset=bass.IndirectOffsetOnAxis(ap=rba[:, 1:2], axis=0))
        nc.gpsimd.tensor_tensor(otmp[:, 0:1], ob[:, 0:1], cc[:, 3:4],
                                OP.add)
        nc.gpsimd.tensor_tensor(otmp[:, 1:2], ob[:, 1:2], cc[:, 4:5],
                                OP.add)
        nc.gpsimd.tensor_tensor(otmp[:, 2:3], ob[:, 2:3], cc[:, 5:6],
                                OP.add)

        # ---- mr tree (DVE) + planes + per-plane DMAs ---------------
        nc.vector.match_replace(etiles[4][:], padk[:, 0:8], e0[:], 0.0)
        nc.vector.match_replace(etiles[8][:], padk[:, 8:16], etiles[4][:],
                                0.0)
        nc.vector.match_replace(etiles[12][:], padk[:, 16:24],
                                etiles[8][:], 0.0)

        def plane(j):
            src = etiles[4 * (j // 4)]
            dst = softs_sb[:, j * FREE:(j + 1) * FREE]
            scl = vbr[:, 16 + j:17 + j]
            eng = PLANE_ENG[j]
            if eng == 'act':
                nc.scalar.activation(dst, src[:], AF.Copy, scale=scl)
            elif eng == 'dve':
                nc.vector.tensor_scalar(dst, src[:], scl, None, OP.mult)
            else:
                nc.gpsimd.tensor_scalar(dst, src[:], scl, None, OP.mult)

        def plane_dma(j):
            tgt = slo_2d if j < 8 else shi_2d
            toff = (j % 8) * FREE
            nc.sync.dma_start(out=tgt[:, toff:toff + FREE],
                              in_=softs_sb[:, j * FREE:(j + 1) * FREE])

        # fixup planes first per engine; exact plane (j%4==0) last.
        for j in (1, 2, 3, 0,            # ACT g0
                  5, 6, 7, 4,            # DVE g1
                  9, 8,                  # ACT g2 part
                  10, 11,                # DVE g2 part
                  13, 14, 15, 12):       # DVE g3 (12 exact, last)
            plane(j)
            plane_dma(j)

        # ---- indirect scatters (gps), in token-readiness order -----
        # st: waits zero-fill DMA
        nc.gpsimd.memset(stz[:, 0:1], 0.0)
        nc.gpsimd.tensor_tensor(offd[:, 0:1], otmp[:, 0:1],
                                stz[:, 0:1].bitcast(U32), OP.add)
        nc.gpsimd.indirect_dma_start(
            out=st_d.ap(),
            out_offset=bass.IndirectOffsetOnAxis(ap=offd[:, 0:1], axis=0),
            in_=ones[:], in_offset=None)
        # A: waits fixup-plane DMAs 1,2,3,5,6,7
        for j in (1, 2, 3, 5, 6, 7):
            nc.gpsimd.memset(softs_sb[:, j * FREE:j * FREE + 2], 0.0)
        tokA = [softs_sb[:, j * FREE:j * FREE + 2].bitcast(U32)
                for j in (1, 2, 3, 5, 6, 7)]
        nc.gpsimd.tensor_tensor(offd[:, 1:2], otmp[:, 1:2], tokA[0],
                                OP.add)
        for t in tokA[1:]:
            nc.gpsimd.tensor_tensor(offd[:, 1:2], offd[:, 1:2], t, OP.add)
        nc.gpsimd.indirect_dma_start(
            out=slo_d.ap(),
            out_offset=bass.IndirectOffsetOnAxis(ap=offd[:, 1:2], axis=0),
            in_=zbf[:], in_offset=None)
        # B: waits fixup-plane DMAs 9,10,11,13,14,15 (NOT 12)
        for j in (9, 10, 11, 13, 14, 15):
            nc.gpsimd.memset(softs_sb[:, j * FREE:j * FREE + 2], 0.0)
        tokB = [softs_sb[:, j * FREE:j * FREE + 2].bitcast(U32)
                for j in (9, 10, 11, 13, 14, 15)]
        nc.gpsimd.tensor_tensor(offd[:, 2:3], otmp[:, 2:3], tokB[0],
                                OP.add)
        for t in tokB[1:]:
            nc.gpsimd.tensor_tensor(offd[:, 2:3], offd[:, 2:3], t, OP.add)
        nc.gpsimd.indirect_dma_start(
            out=shi_d.ap(),
            out_offset=bass.IndirectOffsetOnAxis(ap=offd[:, 2:3], axis=0),
            in_=zbf[:], in_offset=None)
    nc.compile()
    return nc


def kernel(logits, gumbel, k, trace=False):
    K = int(k)
    logits = np.ascontiguousarray(logits, dtype=np.float32)
    gumbel = np.ascontiguousarray(gumbel, dtype=np.float32)
    if K == 0:
        empty = np.zeros((0, B, N), dtype=np.float32)
        return empty, empty.copy()
    assert K == 16, f"kernel supports k=16 only, got {K}"
    assert logits.shape == (B, N) and gumbel.shape == (B, N)

    if K not in _module_cache:
        _module_cache[K] = _build16()
    nc = _module_cache[K]

    cc = _host_consts().view(np.float32)
    z_full = logits + gumbel
    in_maps = []
    for c in range(NCORES):
        sl = slice(c * R, (c + 1) * R)
        zc = np.concatenate([z_full[sl].reshape(P, FREE), cc], axis=1)
        in_maps.append({"zc": np.ascontiguousarray(zc)})

    res = run_bass_kernel_spmd(nc, in_maps, core_ids=list(range(NCORES)),
                               trace=trace)

    st = np.empty((K, B, N), dtype=np.float32)
    softs = np.empty((K, B, N), dtype=np.float32)
    for c in range(NCORES):
        sl = slice(c * R, (c + 1) * R)
        lo = res.results[c]["slo"].reshape(R, QP, 8, FREE)
        hi = res.results[c]["shi"].reshape(R, QP, 8, FREE)
        s = np.concatenate([lo, hi], axis=2)
        softs[:, sl, :] = np.transpose(s.astype(np.float32), (2, 0, 1, 3)) \
            .reshape(K, R, N)
        t = res.results[c]["st"].reshape(R, QP, K16, FREE)
        st[:, sl, :] = np.transpose(t, (2, 0, 1, 3)).reshape(K16, R, N) \
            .astype(np.float32)

    if trace:
        kernel.last_exec_time_ns = res.exec_time_ns
        kernel.last_results = res
    return st, softs
